# revision 52
# baseline (speedup 1.0000x reference)
"""BERT-CRF loss kernel for Trainium2 (8 NeuronCores, data-parallel over positions).

Math: loss = sum_b(forward_b - cumsum(gold)_b) for a CRF whose forward scan runs
over the flattened B*S steps (batch carryover).  The log-semiring scan is
reassociated into per-chunk (L=4 positions) transfer matrices computed on
device in scaled probability space:

  feats[pos,t]  = hidden @ W.T            (PE, pos-major: tags on the free dim
                                           so each matmul costs only 12 rows;
                                           the bias b is folded into the
                                           transition matrix E' = diag(e^b) E
                                           on the host, so no bias add at all)
  mneg[pos]     = -max_t feats[pos,0:10]  (DVE reduce, batched 4 tiles/op)
  EF            = exp(feats - m)          (ACT, bias = mneg per partition)
  chunk scan    : A <- diag(EF_s) @ (E' @ A)

The scan runs in 4 independent passes of 4 sequential steps each. A pass
covers 8 position tiles (slots) stacked 12 rows apiece on 96 partitions; the
E' matmul is ONE block-diagonal [96x96] @ [96x384] per step, and the diag(EF)
is ONE DVE broadcast-multiply. EF is transposed tag-major with a single PE
transpose [128,96]->[96,128] per pass. Dead tags (START/STOP) ride along as
structurally-zero rows.

Host combines the 8192 tiny [12,12] chunk matrices sequentially in f64
(log-semiring matvec), reads off sentence-end vectors, and computes the gold
score from the shipped feats.

Per core: 8 sentences = 4096 positions; hidden arrives pre-transposed
[768, 4096] (fp8 e4m3 by default) so the h-contraction sits on partitions.
"""
import numpy as np
import ml_dtypes
from contextlib import ExitStack

import concourse.bass as bass
import concourse.mybir as mybir
from concourse.tile import TileContext
from concourse.tile_rust import add_dep_helper
from concourse.bass_utils import run_bass_kernel_spmd

B, S, H, T = 64, 512, 768, 12
START, STOP, NEG = 10, 11, -10000.0
NCORES = 8
P_CORE = B * S // NCORES     # 4096 positions per core
L = 4                        # chunk length (positions per transfer matrix)
NCH = P_CORE // L            # 1024 chunks per core
NPASS = 4                    # scan passes per core
TPP = 8                      # position tiles (slots) per pass
NBLK = 8                     # input DMA blocks (512 positions each)
CPS = 128 // L               # chunks per slot (32)

BF16 = ml_dtypes.bfloat16
FP8 = ml_dtypes.float8_e4m3
F32 = mybir.dt.float32
BF = mybir.dt.bfloat16

HID_DT = mybir.dt.float8e4   # device dtype for hidden/W
HID_NP = FP8                 # matching numpy dtype


def _build_nc():
    nc = bass.Bass()
    hiddenT = nc.declare_dram_parameter("hiddenT", [H, P_CORE], HID_DT,
                                        isOutput=False)
    wt = nc.declare_dram_parameter("wt", [H, T], HID_DT, isOutput=False)
    cpack = nc.declare_dram_parameter("cpack", [128, 544], BF, isOutput=False)
    feats_out = nc.declare_dram_parameter("feats_out", [128, 384], BF,
                                          isOutput=True)
    m_out = nc.declare_dram_parameter("m_out", [128, 32], F32, isOutput=True)
    a_out = nc.declare_dram_parameter("a_out", [NPASS * 96, 320], BF,
                                      isOutput=True)

    last = {}
    out_dmas = []
    in_dmas = []

    with ExitStack() as ctx:
        tc = ctx.enter_context(TileContext(nc))
        const_pool = ctx.enter_context(tc.tile_pool(name="const", bufs=1))
        hid_pool = ctx.enter_context(tc.tile_pool(name="hid", bufs=1))
        efpm_pool = ctx.enter_context(tc.tile_pool(name="efpm", bufs=NPASS))
        ef40_pool = ctx.enter_context(tc.tile_pool(name="ef40", bufs=NPASS))
        a_pool = ctx.enter_context(tc.tile_pool(name="apool", bufs=16))
        psf_pool = ctx.enter_context(tc.tile_pool(name="psf", bufs=1,
                                                  space="PSUM"))
        pss_pool = ctx.enter_context(tc.tile_pool(name="pss", bufs=3,
                                                  space="PSUM"))
        pst_pool = ctx.enter_context(tc.tile_pool(name="pst", bufs=2,
                                                  space="PSUM"))
        warm_pool = ctx.enter_context(tc.tile_pool(name="warm", bufs=1,
                                                   space="PSUM"))

        # ---- inputs. Rings: the 8 HWDGE rings go to the 8 SP-queue DMAs
        # (2 inputs + 6 outputs, each needing at most one data wait); all
        # remaining inputs ride the Pool SWDGE queue with zero data waits so
        # their ring-reuse waits are their only wait.
        def hid_dma(engine, pos0, npos):
            t = hid_pool.tile([128, 6 * npos], HID_DT, name=f"hid_{pos0}",
                              tag=f"hid{pos0}")
            di = engine.dma_start(
                out=t[:, :].rearrange("p (k c) -> p k c", c=npos),
                in_=hiddenT[:, pos0:pos0 + npos].rearrange(
                    "(k p) c -> p k c", p=128),
            )
            in_dmas.append(di)
            return t

        h_sp0 = hid_dma(nc.sync, 0, 512)
        h_sp1 = hid_dma(nc.sync, 512, 512)

        wt_sb = const_pool.tile([128, 6 * T], HID_DT)
        di = nc.gpsimd.dma_start(
            out=wt_sb[:, :].rearrange("p (k t) -> p k t", t=T),
            in_=wt[:, :].rearrange("(k p) t -> p k t", p=128),
        )
        in_dmas.append(di)
        cpack_sb = const_pool.tile([128, 544], BF)
        di = nc.gpsimd.dma_start(out=cpack_sb[:, :], in_=cpack[:, :])
        in_dmas.append(di)
        bd_sb = cpack_sb[0:96, 0:96]
        e40_sb = cpack_sb[0:96, 96:416]
        ident_sb = cpack_sb[:, 416:544]

        fsub_big = const_pool.tile([128, 384], BF)
        m_all = const_pool.tile([128, 32], F32)

        h2 = hid_dma(nc.gpsimd, 1024, 512)
        h3 = hid_dma(nc.gpsimd, 1536, 512)
        h45 = hid_dma(nc.gpsimd, 2048, 1024)
        h67 = hid_dma(nc.gpsimd, 3072, 1024)
        # (tile, base column within each k-chunk, k-chunk stride) per block
        hid_of = [(h_sp0, 0, 512), (h_sp1, 0, 512), (h2, 0, 512), (h3, 0, 512),
                  (h45, 0, 1024), (h45, 512, 1024), (h67, 0, 1024),
                  (h67, 512, 1024)]

        # ---- warm-up touches: absorb const-DMA waits + load the ACT
        # exp/copy table once.
        wp = warm_pool.tile([128, 128], F32)
        nc.tensor.matmul(wp[0:T, 0:T], lhsT=wt_sb[:, 0:T], rhs=wt_sb[:, 0:T],
                         start=True, stop=True)
        nc.tensor.matmul(wp[0:96, 0:96], lhsT=bd_sb, rhs=bd_sb,
                         start=True, stop=True)
        nc.tensor.matmul(wp[0:128, 0:1], lhsT=ident_sb,
                         rhs=ident_sb[:, 0:1], start=True, stop=True)
        scr_v = const_pool.tile([1, 8], BF)
        nc.vector.tensor_copy(scr_v[0:1, 0:1], e40_sb[0:1, 0:1])
        scr_a = const_pool.tile([1, 16], F32)
        nc.scalar.activation(scr_a[0:1, 0:1], scr_v[0:1, 0:1],
                             mybir.ActivationFunctionType.Exp)

        psf_all = psf_pool.tile([128, 384], F32)  # 8 x 48-col regions
        ef_pm = [efpm_pool.tile([128, 96], BF, name=f"efpm_{p}", tag="efpm")
                 for p in range(NPASS)]
        ef40 = [ef40_pool.tile([96, 128], BF, name=f"ef40_{p}", tag="ef40")
                for p in range(NPASS)]

        blk_sub = [None] * NBLK
        blk_last_mm = [None] * NBLK

        def feats_block(blk):
            """512 positions: 24 matmuls, DVE reduce + m-subtract, 1 exp."""
            # one private 192B psf region per block (all in one PSUM bank):
            # no buffer reuse, so no WAR/WAW absorbers are needed at all
            psf = psf_all[:, blk * 48:(blk + 1) * 48]
            ht, base, seg = hid_of[blk]
            for pt in range(4):
                for k in range(6):
                    mmi = nc.tensor.matmul(
                        psf[:, pt * T:(pt + 1) * T],
                        lhsT=ht[:, k * seg + base + pt * 128:
                                k * seg + base + (pt + 1) * 128],
                        rhs=wt_sb[:, k * T:(k + 1) * T],
                        start=(k == 0), stop=(k == 5),
                        skip_group_check=True,
                    )
            blk_last_mm[blk] = mmi
            dab = dve_absorb([mmi])
            # -max over live tags (DVE, straight from PSUM, 4 tiles at once)
            ri = nc.vector.tensor_reduce(
                out=m_all[:, blk * 4:(blk + 1) * 4],
                in_=psf[:, :].rearrange(
                    "p (g t) -> p g t", t=T)[:, :, 0:10],
                op=mybir.AluOpType.max, axis=mybir.AxisListType.X,
                negate=True,
            )
            add_dep_helper(ri.ins, dab.ins, False, "pin dve absorber")
            last["red"] = ri
            # fsub = feats - m (DVE in-order after the reduce); doubles as the
            # host-shipped feats (host adds m back)
            m_sl = m_all[:, blk * 4:(blk + 1) * 4]
            m_ap = bass.AP(m_sl.tensor, m_sl.offset,
                           list(m_sl.ap) + [[0, T]])    # [128, 4, 12] bcast
            si = nc.vector.tensor_add(
                fsub_big[:, blk * 48:(blk + 1) * 48].rearrange(
                    "p (g t) -> p g t", t=T),
                psf[:, :].rearrange("p (g t) -> p g t", t=T),
                m_ap,
            )
            blk_sub[blk] = si
            last["sub"] = si
            # ONE exp for the whole block (bias pre-applied above)
            p, slot0 = blk // 2, (blk % 2) * 4
            ei = nc.scalar.activation(
                ef_pm[p][:, slot0 * T:slot0 * T + 48].rearrange(
                    "p (s t) -> p s t", t=T)[:, :, 0:10],
                fsub_big[:, blk * 48:(blk + 1) * 48].rearrange(
                    "p (s t) -> p s t", t=T)[:, :, 0:10],
                mybir.ActivationFunctionType.Exp,
            )
            pass_deps[p].append(ei)
            last["exp"] = ei

        pass_deps = [[] for _ in range(NPASS)]

        def scan_pass(p):
            # zero the dead-tag columns on ACT (in-order with the exps, so the
            # pass transpose needs only the single ACT wait)
            zi = nc.scalar.activation(
                ef_pm[p][:, :].rearrange("p (s t) -> p s t", t=T)[:, :, 10:12],
                ident_sb[:, 0:16].rearrange("p (s t) -> p s t", t=2),
                mybir.ActivationFunctionType.Copy, scale=0.0)
            pass_deps[p].append(zi)

        ab_col = [96]

        def pe_absorb(src=None, deps=()):
            """Tiny matmul that takes a cross-engine wait into the PE clock so
            the following real matmul needs only its own-engine wait (the MM
            struct allows a single sync wait). With deps, reads a const and
            carries explicit dep edges instead of data deps."""
            c = ab_col[0] = 96 + (ab_col[0] - 96 + 1) % 31
            if src is None:
                src = ident_sb
            ab = nc.tensor.matmul(wp[0:1, c:c + 1], lhsT=src[0:1, 0:1],
                                  rhs=src[0:1, 0:1], start=True, stop=True,
                                  skip_group_check=True)
            for d in deps:
                add_dep_helper(ab.ins, d.ins, True, "absorb")
            return ab

        At_cur = [None] * NPASS

        dv_col = [0]

        def dve_absorb(deps):
            """Tiny DVE copy that takes a cross-engine wait into the DVE
            clock (DVE struct also allows a single sync wait)."""
            c = dv_col[0] = (dv_col[0] + 1) % 7
            ab = nc.vector.tensor_copy(scr_v[0:1, c:c + 1], e40_sb[0:1, 0:1])
            for d in deps:
                add_dep_helper(ab.ins, d.ins, True, "dve absorb")
            return ab

        def scan_setup(p):
            # transpose EF tag-major on PE, then one DVE copy PSUM->SBUF.
            # ab1 takes the cross-engine (ACT exps) wait; the pin edge keeps
            # it scheduled right before the transpose.
            pst = pst_pool.tile([96, 1024], BF, name=f"pst_{p}", tag="pst")
            ab1 = pe_absorb(deps=pass_deps[p])
            ti = nc.tensor.transpose(pst[:, 0:128], ef_pm[p][:, :],
                                     ident_sb)
            add_dep_helper(ti.ins, ab1.ins, False, "pin absorber")
            dab = dve_absorb(pass_deps[p])
            cpi = nc.vector.tensor_copy(ef40[p][:, :], pst[:, 0:128])
            add_dep_helper(cpi.ins, dab.ins, False, "pin dve absorber")

        def scan_step(p, s):
            # A carries only the 10 live input columns (j=START
            # contributions are a host-side special case for chunk 0).
            ef_base = ef40[p][:, s::L]                    # [96, 32]
            ef_ap = bass.AP(ef_base.tensor, ef_base.offset,
                            list(ef_base.ap) + [[0, 10]])  # [96, 32, 10]
            At2 = a_pool.tile([96, 320], BF, name=f"at_{p}_{s}", tag="at")
            if s == 0:
                vi = nc.vector.tensor_mul(
                    At2[:, :].rearrange("p (c j) -> p c j", j=10),
                    e40_sb.rearrange("p (c j) -> p c j", j=10),
                    ef_ap,
                )
            else:
                ps = pss_pool.tile([96, 512], F32)       # full bank; 320 used
                sab = pe_absorb(At_cur[p])
                mi = nc.tensor.matmul(ps[:, 0:320], lhsT=bd_sb,
                                      rhs=At_cur[p][:, :], start=True,
                                      stop=True)
                add_dep_helper(mi.ins, sab.ins, False, "pin absorber")
                last["pe"] = mi
                vi = nc.vector.tensor_mul(
                    At2[:, :].rearrange("p (c j) -> p c j", j=10),
                    ps[:, 0:320].rearrange("p (c j) -> p c j", j=10),
                    ef_ap,
                )
            last["dve"] = vi
            At_cur[p] = At2

        def scan_ship(p):
            oi = nc.sync.dma_start(out=a_out[p * 96:(p + 1) * 96, :],
                                   in_=At_cur[p][:, :])
            out_dmas.append(oi)

        def feats_m_ship():
            oi = nc.sync.dma_start(out=feats_out[:, :], in_=fsub_big[:, :])
            out_dmas.append(oi)
            oi = nc.sync.dma_start(out=m_out[:, :], in_=m_all[:, :])
            out_dmas.append(oi)

        for p in range(NPASS):
            scan_pass(p)          # dead-col zero writes early (ACT, cheap)

        # Data-arrival-aware interleave: blocks land every ~1.1-2.4us; scan
        # steps of adjacent passes alternate so the DVE queue stays busy while
        # each pass waits on its PE matmul round-trips.
        B, SS = feats_block, scan_setup
        ST, SH = scan_step, scan_ship
        B(0); B(1); SS(0)
        B(2); ST(0, 0)
        B(3); ST(0, 1); SS(1)
        ST(1, 0); ST(0, 2)
        ST(1, 1); ST(0, 3); SH(0)
        B(4); B(5); ST(1, 2); SS(2)
        ST(1, 3); SH(1)
        B(6); B(7); ST(2, 0); SS(3)
        feats_m_ship()
        ST(3, 0); ST(2, 1)
        ST(3, 1); ST(2, 2)
        ST(3, 2); ST(2, 3); SH(2)
        ST(3, 3); SH(3)
        # Pre-absorb every proc's clock into SP one dep at a time, so the
        # Tile tail drain does not need a multi-sem wait.
        for dep in in_dmas + out_dmas + list(last.values()):
            nop = nc.sync.nop()
            add_dep_helper(nop.ins, dep.ins, True, "drain preclear")
    return nc


_NC_CACHE = None


def _get_nc():
    global _NC_CACHE
    if _NC_CACHE is None:
        _NC_CACHE = _build_nc()
    return _NC_CACHE


def _build_eprime(transitions, b):
    """E' = diag(e^b) exp(transitions) with structurally-dead rows/cols zeroed."""
    E = np.exp(transitions.astype(np.float64))
    E[START, :] = 0.0
    E[STOP, :] = 0.0
    E[:, STOP] = 0.0
    E = E * np.exp(b.astype(np.float64))[:, None]
    return E


def _build_bd(Ep):
    """Block-diagonal stationary operand: bd[slot*12+p, slot*12+t'] = E'[t',p]."""
    bd = np.zeros((96, 96), np.float64)
    for s in range(TPP):
        bd[s * T:(s + 1) * T, s * T:(s + 1) * T] = Ep.T
    return bd.astype(BF16)


def _build_cpack(Ep):
    """Packed bf16 consts: block-diag E'^T, e40 (live j), ident."""
    pack = np.zeros((128, 544), BF16)
    pack[0:96, 0:96] = _build_bd(Ep)
    pack[0:96, 96:416] = _build_e40(Ep)
    pack[:, 416:544] = np.eye(128).astype(BF16)
    return pack


def _build_e40(Ep):
    """Step-0 init: e40[slot*12+t, c*10+j] = E'[t, j] (live j only)."""
    e = np.zeros((96, 320), np.float64)
    tile = np.tile(Ep[:, 0:10], (1, CPS))
    for s in range(TPP):
        e[s * T:(s + 1) * T, :] = tile
    return e.astype(BF16)


def _sim_input_map(inputs, core):
    """Per-core device input map (also used by test harnesses)."""
    hidden = np.asarray(inputs["hidden"], dtype=np.float32)
    W = np.asarray(inputs["W"], dtype=np.float32)
    b = np.asarray(inputs["b"], dtype=np.float32)
    transitions = np.asarray(inputs["transitions"], dtype=np.float32)
    Ep = _build_eprime(transitions, b)
    flat = hidden.reshape(B * S, H)
    hT = np.ascontiguousarray(
        flat[core * P_CORE:(core + 1) * P_CORE].T).astype(HID_NP)
    return {
        "hiddenT": hT,
        "wt": np.ascontiguousarray(W.T).astype(HID_NP),
        "cpack": _build_cpack(Ep),
    }


def _run_device(hidden, W, b, transitions, trace=False, tmpdir=None):
    Ep = _build_eprime(transitions, b)
    in_common = {
        "wt": np.ascontiguousarray(W.T).astype(HID_NP),
        "cpack": _build_cpack(Ep),
    }
    flat = hidden.reshape(B * S, H)
    in_maps = []
    for c in range(NCORES):
        hT = np.ascontiguousarray(
            flat[c * P_CORE:(c + 1) * P_CORE].T).astype(HID_NP)
        d = dict(in_common)
        d["hiddenT"] = hT
        in_maps.append(d)

    res = run_bass_kernel_spmd(
        _get_nc(), in_maps, list(range(NCORES)), trace=trace, tmpdir=tmpdir)
    return res


def _host_combine(results, transitions, b, tags):
    # fsub = feats - m [B*S, T] (bf16 -> f64), WITHOUT the bias b
    feats = np.concatenate([
        np.asarray(r["feats_out"]).astype(np.float64)
        .reshape(128, 32, T).transpose(1, 0, 2).reshape(P_CORE, T)
        for r in results], axis=0)
    # m per position (device ships -m)
    m_flat = np.concatenate([
        -np.asarray(r["m_out"]).astype(np.float64).T.reshape(P_CORE)
        for r in results])
    feats = feats + m_flat[:, None]    # reconstruct raw feats
    # chunk matrices [NCORES*NCH, 12, 10] (live input columns only)
    A = np.concatenate([
        np.asarray(r["a_out"]).astype(np.float64)
        .reshape(NPASS, TPP, T, CPS, 10).transpose(0, 1, 3, 2, 4)
        .reshape(NCH, T, 10)
        for r in results], axis=0)
    n_chunks = NCORES * NCH
    scale = m_flat.reshape(n_chunks, L).sum(axis=1)
    with np.errstate(divide="ignore"):
        logP = np.log(A) + scale[:, None, None]

    # chunk 0 exactly on the host (the only chunk whose START column matters)
    tr64 = transitions.astype(np.float64)
    f0 = feats[0:L] + b.astype(np.float64)[None, :]
    v = tr64[:, START] + f0[0]
    for s in range(1, L):
        xx = v[None, :] + tr64
        mxx = xx.max(axis=1)
        v = mxx + np.log(np.exp(xx - mxx[:, None]).sum(axis=1)) + f0[s]

    last = np.zeros((B, T), np.float64)
    cps_sentence = S // L
    err = np.errstate(invalid="ignore", divide="ignore", over="ignore")
    err.__enter__()
    for c in range(1, n_chunks):
        x = logP[c] + v[None, 0:10]
        mx = np.max(x, axis=1)
        mx_safe = np.where(np.isfinite(mx), mx, 0.0)
        vl = mx + np.log(np.sum(np.exp(x - mx_safe[:, None]), axis=1))
        v = np.where(np.isfinite(mx), vl, -np.inf)
        if (c + 1) % cps_sentence == 0:
            last[(c + 1) // cps_sentence - 1] = v
    x = last + transitions[STOP][None, :].astype(np.float64)
    mx = x.max(axis=1)
    forward_score = mx + np.log(np.exp(x - mx[:, None]).sum(axis=1))  # [B]
    err.__exit__(None, None, None)

    tags_ext = np.concatenate(
        [np.full((B, 1), START, dtype=tags.dtype), tags], axis=1)
    prev, nxt = tags_ext[:, :-1], tags_ext[:, 1:]
    trans_sc = transitions[nxt, prev].astype(np.float64).sum(axis=1)
    featsb = feats.reshape(B, S, T)
    emit_sc = np.take_along_axis(
        featsb, nxt[..., None].astype(np.int64), axis=2)[..., 0].sum(axis=1)
    emit_sc = emit_sc + b.astype(np.float64)[nxt].sum(axis=1)
    gold = trans_sc + emit_sc + transitions[STOP, tags_ext[:, -1]].astype(np.float64)
    gold_cum = np.cumsum(gold)
    out = np.sum(forward_score - gold_cum)
    return np.array([out], dtype=np.float32)


def kernel(hidden, W, b, transitions, tags, _trace=False, _tmpdir=None):
    hidden = np.asarray(hidden, dtype=np.float32)
    W = np.asarray(W, dtype=np.float32)
    b = np.asarray(b, dtype=np.float32)
    transitions = np.asarray(transitions, dtype=np.float32)
    tags = np.asarray(tags)
    res = _run_device(hidden, W, b, transitions, trace=_trace, tmpdir=_tmpdir)
    out = _host_combine(res.results, transitions, b, tags)
    if _trace:
        return out, res
    return out


# revision 53
# speedup vs baseline: 1.0043x; 1.0043x over previous
"""BERT-CRF loss kernel for Trainium2 (8 NeuronCores, data-parallel over positions).

Math: loss = sum_b(forward_b - cumsum(gold)_b) for a CRF whose forward scan runs
over the flattened B*S steps (batch carryover).  The log-semiring scan is
reassociated into per-chunk (L=4 positions) transfer matrices computed on
device in scaled probability space:

  feats[pos,t]  = hidden @ W.T            (PE, pos-major: tags on the free dim
                                           so each matmul costs only 12 rows;
                                           the bias b is folded into the
                                           transition matrix E' = diag(e^b) E
                                           on the host, so no bias add at all)
  mneg[pos]     = -max_t feats[pos,0:10]  (DVE reduce, batched 4 tiles/op)
  EF            = exp(feats - m)          (ACT, bias = mneg per partition)
  chunk scan    : A <- diag(EF_s) @ (E' @ A)

The scan runs in 4 independent passes of 4 sequential steps each. A pass
covers 8 position tiles (slots) stacked 12 rows apiece on 96 partitions; the
E' matmul is ONE block-diagonal [96x96] @ [96x384] per step, and the diag(EF)
is ONE DVE broadcast-multiply. EF is transposed tag-major with a single PE
transpose [128,96]->[96,128] per pass. Dead tags (START/STOP) ride along as
structurally-zero rows.

Host combines the 8192 tiny [12,12] chunk matrices sequentially in f64
(log-semiring matvec), reads off sentence-end vectors, and computes the gold
score from the shipped feats.

Per core: 8 sentences = 4096 positions; hidden arrives pre-transposed
[768, 4096] (fp8 e4m3 by default) so the h-contraction sits on partitions.
"""
import numpy as np
import ml_dtypes
from contextlib import ExitStack

import concourse.bass as bass
import concourse.mybir as mybir
from concourse.tile import TileContext
from concourse.tile_rust import add_dep_helper
from concourse.bass_utils import run_bass_kernel_spmd

B, S, H, T = 64, 512, 768, 12
START, STOP, NEG = 10, 11, -10000.0
NCORES = 8
P_CORE = B * S // NCORES     # 4096 positions per core
L = 4                        # chunk length (positions per transfer matrix)
NCH = P_CORE // L            # 1024 chunks per core
NPASS = 4                    # scan passes per core
TPP = 8                      # position tiles (slots) per pass
NBLK = 8                     # input DMA blocks (512 positions each)
CPS = 128 // L               # chunks per slot (32)

BF16 = ml_dtypes.bfloat16
FP8 = ml_dtypes.float8_e4m3
F32 = mybir.dt.float32
BF = mybir.dt.bfloat16

HID_DT = mybir.dt.float8e4   # device dtype for hidden/W
HID_NP = FP8                 # matching numpy dtype


def _build_nc():
    nc = bass.Bass()
    hiddenT = nc.declare_dram_parameter("hiddenT", [H, P_CORE], HID_DT,
                                        isOutput=False)
    wt = nc.declare_dram_parameter("wt", [H, T], HID_DT, isOutput=False)
    cpack = nc.declare_dram_parameter("cpack", [128, 544], BF, isOutput=False)
    feats_out = nc.declare_dram_parameter("feats_out", [128, 384], BF,
                                          isOutput=True)
    m_out = nc.declare_dram_parameter("m_out", [128, 32], F32, isOutput=True)
    a_out = nc.declare_dram_parameter("a_out", [NPASS * 96, 320], BF,
                                      isOutput=True)

    last = {}
    out_dmas = []
    in_dmas = []

    with ExitStack() as ctx:
        tc = ctx.enter_context(TileContext(nc))
        const_pool = ctx.enter_context(tc.tile_pool(name="const", bufs=1))
        hid_pool = ctx.enter_context(tc.tile_pool(name="hid", bufs=1))
        efpm_pool = ctx.enter_context(tc.tile_pool(name="efpm", bufs=NPASS))
        ef40_pool = ctx.enter_context(tc.tile_pool(name="ef40", bufs=NPASS))
        a_pool = ctx.enter_context(tc.tile_pool(name="apool", bufs=16))
        psf_pool = ctx.enter_context(tc.tile_pool(name="psf", bufs=1,
                                                  space="PSUM"))
        pss_pool = ctx.enter_context(tc.tile_pool(name="pss", bufs=3,
                                                  space="PSUM"))
        pst_pool = ctx.enter_context(tc.tile_pool(name="pst", bufs=2,
                                                  space="PSUM"))
        warm_pool = ctx.enter_context(tc.tile_pool(name="warm", bufs=1,
                                                   space="PSUM"))

        # ---- inputs. Rings: the 8 HWDGE rings go to the 8 SP-queue DMAs
        # (2 inputs + 6 outputs, each needing at most one data wait); all
        # remaining inputs ride the Pool SWDGE queue with zero data waits so
        # their ring-reuse waits are their only wait.
        def hid_dma(engine, pos0, npos):
            t = hid_pool.tile([128, 6 * npos], HID_DT, name=f"hid_{pos0}",
                              tag=f"hid{pos0}")
            di = engine.dma_start(
                out=t[:, :].rearrange("p (k c) -> p k c", c=npos),
                in_=hiddenT[:, pos0:pos0 + npos].rearrange(
                    "(k p) c -> p k c", p=128),
            )
            in_dmas.append(di)
            return t

        h_sp0 = hid_dma(nc.sync, 0, 512)
        h_sp1 = hid_dma(nc.sync, 512, 512)

        wt_sb = const_pool.tile([128, 6 * T], HID_DT)
        di = nc.gpsimd.dma_start(
            out=wt_sb[:, :].rearrange("p (k t) -> p k t", t=T),
            in_=wt[:, :].rearrange("(k p) t -> p k t", p=128),
        )
        in_dmas.append(di)
        cpack_sb = const_pool.tile([128, 544], BF)
        di = nc.gpsimd.dma_start(out=cpack_sb[:, :], in_=cpack[:, :])
        in_dmas.append(di)
        bd_sb = cpack_sb[0:96, 0:96]
        e40_sb = cpack_sb[0:96, 96:416]
        ident_sb = cpack_sb[:, 416:544]

        fsub_big = const_pool.tile([128, 384], BF)
        m_all = const_pool.tile([128, 32], F32)

        h2 = hid_dma(nc.gpsimd, 1024, 512)
        h3 = hid_dma(nc.gpsimd, 1536, 512)
        h45 = hid_dma(nc.gpsimd, 2048, 1024)
        h67 = hid_dma(nc.gpsimd, 3072, 1024)
        # (tile, base column within each k-chunk, k-chunk stride) per block
        hid_of = [(h_sp0, 0, 512), (h_sp1, 0, 512), (h2, 0, 512), (h3, 0, 512),
                  (h45, 0, 1024), (h45, 512, 1024), (h67, 0, 1024),
                  (h67, 512, 1024)]

        # ---- warm-up touches: absorb const-DMA waits + load the ACT
        # exp/copy table once.
        wp = warm_pool.tile([128, 128], F32)
        nc.tensor.matmul(wp[0:T, 0:T], lhsT=wt_sb[:, 0:T], rhs=wt_sb[:, 0:T],
                         start=True, stop=True)
        nc.tensor.matmul(wp[0:96, 0:96], lhsT=bd_sb, rhs=bd_sb,
                         start=True, stop=True)
        nc.tensor.matmul(wp[0:128, 0:1], lhsT=ident_sb,
                         rhs=ident_sb[:, 0:1], start=True, stop=True)
        scr_v = const_pool.tile([1, 8], BF)
        nc.vector.tensor_copy(scr_v[0:1, 0:1], e40_sb[0:1, 0:1])
        scr_a = const_pool.tile([1, 16], F32)
        nc.scalar.activation(scr_a[0:1, 0:1], scr_v[0:1, 0:1],
                             mybir.ActivationFunctionType.Exp)

        psf_all = psf_pool.tile([128, 384], F32)  # 8 x 48-col regions
        ef_pm = [efpm_pool.tile([128, 96], BF, name=f"efpm_{p}", tag="efpm")
                 for p in range(NPASS)]
        ef40 = [ef40_pool.tile([96, 128], BF, name=f"ef40_{p}", tag="ef40")
                for p in range(NPASS)]

        blk_sub = [None] * NBLK
        blk_last_mm = [None] * NBLK

        def feats_block(blk):
            """512 positions: 24 matmuls, DVE reduce + m-subtract, 1 exp."""
            # one private 192B psf region per block (all in one PSUM bank):
            # no buffer reuse, so no WAR/WAW absorbers are needed at all
            psf = psf_all[:, blk * 48:(blk + 1) * 48]
            ht, base, seg = hid_of[blk]
            for pt in range(4):
                for k in range(6):
                    mmi = nc.tensor.matmul(
                        psf[:, pt * T:(pt + 1) * T],
                        lhsT=ht[:, k * seg + base + pt * 128:
                                k * seg + base + (pt + 1) * 128],
                        rhs=wt_sb[:, k * T:(k + 1) * T],
                        start=(k == 0), stop=(k == 5),
                        skip_group_check=True,
                    )
            blk_last_mm[blk] = mmi
            dab = dve_absorb([mmi])
            # -max over live tags (DVE, straight from PSUM, 4 tiles at once)
            ri = nc.vector.tensor_reduce(
                out=m_all[:, blk * 4:(blk + 1) * 4],
                in_=psf[:, :].rearrange(
                    "p (g t) -> p g t", t=T)[:, :, 0:10],
                op=mybir.AluOpType.max, axis=mybir.AxisListType.X,
                negate=True,
            )
            add_dep_helper(ri.ins, dab.ins, False, "pin dve absorber")
            last["red"] = ri
            # fsub = feats - m (DVE in-order after the reduce); doubles as the
            # host-shipped feats (host adds m back)
            m_sl = m_all[:, blk * 4:(blk + 1) * 4]
            m_ap = bass.AP(m_sl.tensor, m_sl.offset,
                           list(m_sl.ap) + [[0, T]])    # [128, 4, 12] bcast
            si = nc.vector.tensor_add(
                fsub_big[:, blk * 48:(blk + 1) * 48].rearrange(
                    "p (g t) -> p g t", t=T),
                psf[:, :].rearrange("p (g t) -> p g t", t=T),
                m_ap,
            )
            blk_sub[blk] = si
            last["sub"] = si
            # ONE exp for the whole block (bias pre-applied above)
            p, slot0 = blk // 2, (blk % 2) * 4
            ei = nc.scalar.activation(
                ef_pm[p][:, slot0 * T:slot0 * T + 48].rearrange(
                    "p (s t) -> p s t", t=T)[:, :, 0:10],
                fsub_big[:, blk * 48:(blk + 1) * 48].rearrange(
                    "p (s t) -> p s t", t=T)[:, :, 0:10],
                mybir.ActivationFunctionType.Exp,
            )
            pass_deps[p].append(ei)
            last["exp"] = ei

        pass_deps = [[] for _ in range(NPASS)]

        def scan_pass(p):
            # zero the dead-tag columns on ACT (in-order with the exps, so the
            # pass transpose needs only the single ACT wait)
            zi = nc.scalar.activation(
                ef_pm[p][:, :].rearrange("p (s t) -> p s t", t=T)[:, :, 10:12],
                ident_sb[:, 0:16].rearrange("p (s t) -> p s t", t=2),
                mybir.ActivationFunctionType.Copy, scale=0.0)
            pass_deps[p].append(zi)

        ab_col = [96]

        def pe_absorb(src=None, deps=()):
            """Tiny matmul that takes a cross-engine wait into the PE clock so
            the following real matmul needs only its own-engine wait (the MM
            struct allows a single sync wait). With deps, reads a const and
            carries explicit dep edges instead of data deps."""
            c = ab_col[0] = 96 + (ab_col[0] - 96 + 1) % 31
            if src is None:
                src = ident_sb
            ab = nc.tensor.matmul(wp[0:1, c:c + 1], lhsT=src[0:1, 0:1],
                                  rhs=src[0:1, 0:1], start=True, stop=True,
                                  skip_group_check=True)
            for d in deps:
                add_dep_helper(ab.ins, d.ins, True, "absorb")
            return ab

        At_cur = [None] * NPASS

        dv_col = [0]

        def dve_absorb(deps):
            """Tiny DVE copy that takes a cross-engine wait into the DVE
            clock (DVE struct also allows a single sync wait)."""
            c = dv_col[0] = (dv_col[0] + 1) % 7
            ab = nc.vector.tensor_copy(scr_v[0:1, c:c + 1], e40_sb[0:1, 0:1])
            for d in deps:
                add_dep_helper(ab.ins, d.ins, True, "dve absorb")
            return ab

        def scan_setup(p):
            # transpose EF tag-major on PE, then one DVE copy PSUM->SBUF.
            # ab1 takes the cross-engine (ACT exps) wait; the pin edge keeps
            # it scheduled right before the transpose.
            pst = pst_pool.tile([96, 1024], BF, name=f"pst_{p}", tag="pst")
            ab1 = pe_absorb(deps=pass_deps[p])
            ti = nc.tensor.transpose(pst[:, 0:128], ef_pm[p][:, :],
                                     ident_sb)
            add_dep_helper(ti.ins, ab1.ins, False, "pin absorber")
            dab = dve_absorb(pass_deps[p])
            cpi = nc.vector.tensor_copy(ef40[p][:, :], pst[:, 0:128])
            add_dep_helper(cpi.ins, dab.ins, False, "pin dve absorber")

        def scan_step(p, s):
            # A carries only the 10 live input columns (j=START
            # contributions are a host-side special case for chunk 0).
            ef_base = ef40[p][:, s::L]                    # [96, 32]
            ef_ap = bass.AP(ef_base.tensor, ef_base.offset,
                            list(ef_base.ap) + [[0, 10]])  # [96, 32, 10]
            At2 = a_pool.tile([96, 320], BF, name=f"at_{p}_{s}", tag="at")
            if s == 0:
                vi = nc.vector.tensor_mul(
                    At2[:, :].rearrange("p (c j) -> p c j", j=10),
                    e40_sb.rearrange("p (c j) -> p c j", j=10),
                    ef_ap,
                )
            else:
                ps = pss_pool.tile([96, 512], F32)       # full bank; 320 used
                sab = pe_absorb(At_cur[p])
                mi = nc.tensor.matmul(ps[:, 0:320], lhsT=bd_sb,
                                      rhs=At_cur[p][:, :], start=True,
                                      stop=True)
                add_dep_helper(mi.ins, sab.ins, False, "pin absorber")
                last["pe"] = mi
                vi = nc.vector.tensor_mul(
                    At2[:, :].rearrange("p (c j) -> p c j", j=10),
                    ps[:, 0:320].rearrange("p (c j) -> p c j", j=10),
                    ef_ap,
                )
            last["dve"] = vi
            At_cur[p] = At2

        def scan_ship(p):
            oi = nc.sync.dma_start(out=a_out[p * 96:(p + 1) * 96, :],
                                   in_=At_cur[p][:, :])
            out_dmas.append(oi)

        def feats_m_ship():
            oi = nc.sync.dma_start(out=feats_out[:, :], in_=fsub_big[:, :])
            out_dmas.append(oi)
            oi = nc.sync.dma_start(out=m_out[:, :], in_=m_all[:, :])
            out_dmas.append(oi)

        for p in range(NPASS):
            scan_pass(p)          # dead-col zero writes early (ACT, cheap)

        # Data-arrival-aware interleave: blocks land every ~1.1-2.4us; scan
        # steps of adjacent passes alternate so the DVE queue stays busy while
        # each pass waits on its PE matmul round-trips.
        B, SS = feats_block, scan_setup
        ST, SH = scan_step, scan_ship
        B(0); B(1); SS(0)
        B(2); ST(0, 0)
        B(3); ST(0, 1); SS(1)
        ST(0, 2); ST(1, 0)
        ST(0, 3); SH(0); ST(1, 1)
        B(4); B(5); ST(1, 2); SS(2)
        ST(1, 3); SH(1)
        B(6); B(7); ST(2, 0); SS(3)
        feats_m_ship()
        ST(3, 0); ST(2, 1)
        ST(3, 1); ST(2, 2)
        ST(3, 2); ST(2, 3); SH(2)
        ST(3, 3); SH(3)
        # Pre-absorb every proc's clock into SP one dep at a time, so the
        # Tile tail drain does not need a multi-sem wait.
        for dep in in_dmas + out_dmas + list(last.values()):
            nop = nc.sync.nop()
            add_dep_helper(nop.ins, dep.ins, True, "drain preclear")
    return nc


_NC_CACHE = None


def _get_nc():
    global _NC_CACHE
    if _NC_CACHE is None:
        _NC_CACHE = _build_nc()
    return _NC_CACHE


def _build_eprime(transitions, b):
    """E' = diag(e^b) exp(transitions) with structurally-dead rows/cols zeroed."""
    E = np.exp(transitions.astype(np.float64))
    E[START, :] = 0.0
    E[STOP, :] = 0.0
    E[:, STOP] = 0.0
    E = E * np.exp(b.astype(np.float64))[:, None]
    return E


def _build_bd(Ep):
    """Block-diagonal stationary operand: bd[slot*12+p, slot*12+t'] = E'[t',p]."""
    bd = np.zeros((96, 96), np.float64)
    for s in range(TPP):
        bd[s * T:(s + 1) * T, s * T:(s + 1) * T] = Ep.T
    return bd.astype(BF16)


def _build_cpack(Ep):
    """Packed bf16 consts: block-diag E'^T, e40 (live j), ident."""
    pack = np.zeros((128, 544), BF16)
    pack[0:96, 0:96] = _build_bd(Ep)
    pack[0:96, 96:416] = _build_e40(Ep)
    pack[:, 416:544] = np.eye(128).astype(BF16)
    return pack


def _build_e40(Ep):
    """Step-0 init: e40[slot*12+t, c*10+j] = E'[t, j] (live j only)."""
    e = np.zeros((96, 320), np.float64)
    tile = np.tile(Ep[:, 0:10], (1, CPS))
    for s in range(TPP):
        e[s * T:(s + 1) * T, :] = tile
    return e.astype(BF16)


def _sim_input_map(inputs, core):
    """Per-core device input map (also used by test harnesses)."""
    hidden = np.asarray(inputs["hidden"], dtype=np.float32)
    W = np.asarray(inputs["W"], dtype=np.float32)
    b = np.asarray(inputs["b"], dtype=np.float32)
    transitions = np.asarray(inputs["transitions"], dtype=np.float32)
    Ep = _build_eprime(transitions, b)
    flat = hidden.reshape(B * S, H)
    hT = np.ascontiguousarray(
        flat[core * P_CORE:(core + 1) * P_CORE].T).astype(HID_NP)
    return {
        "hiddenT": hT,
        "wt": np.ascontiguousarray(W.T).astype(HID_NP),
        "cpack": _build_cpack(Ep),
    }


def _run_device(hidden, W, b, transitions, trace=False, tmpdir=None):
    Ep = _build_eprime(transitions, b)
    in_common = {
        "wt": np.ascontiguousarray(W.T).astype(HID_NP),
        "cpack": _build_cpack(Ep),
    }
    flat = hidden.reshape(B * S, H)
    in_maps = []
    for c in range(NCORES):
        hT = np.ascontiguousarray(
            flat[c * P_CORE:(c + 1) * P_CORE].T).astype(HID_NP)
        d = dict(in_common)
        d["hiddenT"] = hT
        in_maps.append(d)

    res = run_bass_kernel_spmd(
        _get_nc(), in_maps, list(range(NCORES)), trace=trace, tmpdir=tmpdir)
    return res


def _host_combine(results, transitions, b, tags):
    # fsub = feats - m [B*S, T] (bf16 -> f64), WITHOUT the bias b
    feats = np.concatenate([
        np.asarray(r["feats_out"]).astype(np.float64)
        .reshape(128, 32, T).transpose(1, 0, 2).reshape(P_CORE, T)
        for r in results], axis=0)
    # m per position (device ships -m)
    m_flat = np.concatenate([
        -np.asarray(r["m_out"]).astype(np.float64).T.reshape(P_CORE)
        for r in results])
    feats = feats + m_flat[:, None]    # reconstruct raw feats
    # chunk matrices [NCORES*NCH, 12, 10] (live input columns only)
    A = np.concatenate([
        np.asarray(r["a_out"]).astype(np.float64)
        .reshape(NPASS, TPP, T, CPS, 10).transpose(0, 1, 3, 2, 4)
        .reshape(NCH, T, 10)
        for r in results], axis=0)
    n_chunks = NCORES * NCH
    scale = m_flat.reshape(n_chunks, L).sum(axis=1)
    with np.errstate(divide="ignore"):
        logP = np.log(A) + scale[:, None, None]

    # chunk 0 exactly on the host (the only chunk whose START column matters)
    tr64 = transitions.astype(np.float64)
    f0 = feats[0:L] + b.astype(np.float64)[None, :]
    v = tr64[:, START] + f0[0]
    for s in range(1, L):
        xx = v[None, :] + tr64
        mxx = xx.max(axis=1)
        v = mxx + np.log(np.exp(xx - mxx[:, None]).sum(axis=1)) + f0[s]

    last = np.zeros((B, T), np.float64)
    cps_sentence = S // L
    err = np.errstate(invalid="ignore", divide="ignore", over="ignore")
    err.__enter__()
    for c in range(1, n_chunks):
        x = logP[c] + v[None, 0:10]
        mx = np.max(x, axis=1)
        mx_safe = np.where(np.isfinite(mx), mx, 0.0)
        vl = mx + np.log(np.sum(np.exp(x - mx_safe[:, None]), axis=1))
        v = np.where(np.isfinite(mx), vl, -np.inf)
        if (c + 1) % cps_sentence == 0:
            last[(c + 1) // cps_sentence - 1] = v
    x = last + transitions[STOP][None, :].astype(np.float64)
    mx = x.max(axis=1)
    forward_score = mx + np.log(np.exp(x - mx[:, None]).sum(axis=1))  # [B]
    err.__exit__(None, None, None)

    tags_ext = np.concatenate(
        [np.full((B, 1), START, dtype=tags.dtype), tags], axis=1)
    prev, nxt = tags_ext[:, :-1], tags_ext[:, 1:]
    trans_sc = transitions[nxt, prev].astype(np.float64).sum(axis=1)
    featsb = feats.reshape(B, S, T)
    emit_sc = np.take_along_axis(
        featsb, nxt[..., None].astype(np.int64), axis=2)[..., 0].sum(axis=1)
    emit_sc = emit_sc + b.astype(np.float64)[nxt].sum(axis=1)
    gold = trans_sc + emit_sc + transitions[STOP, tags_ext[:, -1]].astype(np.float64)
    gold_cum = np.cumsum(gold)
    out = np.sum(forward_score - gold_cum)
    return np.array([out], dtype=np.float32)


def kernel(hidden, W, b, transitions, tags, _trace=False, _tmpdir=None):
    hidden = np.asarray(hidden, dtype=np.float32)
    W = np.asarray(W, dtype=np.float32)
    b = np.asarray(b, dtype=np.float32)
    transitions = np.asarray(transitions, dtype=np.float32)
    tags = np.asarray(tags)
    res = _run_device(hidden, W, b, transitions, trace=_trace, tmpdir=_tmpdir)
    out = _host_combine(res.results, transitions, b, tags)
    if _trace:
        return out, res
    return out
